# revision 63
# baseline (speedup 1.0000x reference)
"""Llama GQA attention (B=1, S=2048, E=4096, H=32, KV=8, D=128) on 8 trn2 cores.

Sharding: tensor-parallel over KV groups. Core c owns kv head c and q heads
4c..4c+3: wq/wk/wv output-dim shards, wo input-dim shard. Each core computes a
partial [S, E] output (fp16); host sums the 8 partials and adds bo.

Device layout (per core): activations transposed [feature, seq], fp16 operands
(bf16 for the softmax P / V-tiles), fp32 PSUM accumulation:
  phase 1 per seq-chunk of 512, two passes over the same x tiles so PSUM
  drains (RoPE on DVE, V transpose via PE) overlap the next pass's matmuls:
    pass A: k, v, q0, q1 accumulators (PSUM banks 0-3)
    pass B: q2, q3 (banks 4-5); V transposes in banks 6-7.
  phase 2 per (q-chunk, head): scores^T tile [k 128, q 512] = kr.T-matmul;
  exp on ACT -> P bf16; causal masking of diagonal tiles via affine_select
  (no mask DMAs); AV + row-sum(ones) matmuls accumulate in PSUM.
  phase 3: o-proj per 128-row q chunk into a [128, 4096] fp16 staging tile,
  one DMA store per chunk issued from the ACT engine.
All weights are host-packed into their SBUF layout so every load is a
contiguous full-bandwidth DMA; ~48 DMAs total.
"""

import sys

sys.path.insert(0, "/opt/trn_rl_repo")

import numpy as np
import ml_dtypes

import concourse.bass as bass  # noqa: F401
import concourse.bacc as bacc
import concourse.mybir as mybir
import concourse.tile as tile
from concourse.bass_utils import run_bass_kernel_spmd
from concourse.masks import make_identity

F32 = mybir.dt.float32
F16 = mybir.dt.float16
BF16 = mybir.dt.bfloat16
ADD = mybir.AluOpType.add
MULT = mybir.AluOpType.mult
IS_GE = mybir.AluOpType.is_ge
EXP = mybir.ActivationFunctionType.Exp

NPF16 = np.float16
NPBF16 = ml_dtypes.bfloat16

B, S, E = 1, 2048, 4096
H, KV, D = 32, 8, 128
NCORES = 8
HPC = H // NCORES          # 4 q heads per core
ET = E // 128              # 32 contraction tiles
SC = S // 512              # 4 seq chunks of 512
KT = S // 128              # 16 k tiles of 128
ECH = E // 512             # 8 output E chunks

# per-(qc, kt) tile modes
SKIP, NOMASK, DIAG, DMASK = 0, 1, 2, 3

_build_cache = {}


def _build(modes, use_bias):
    nc = bacc.Bacc(None, target_bir_lowering=False)

    xT = nc.declare_dram_parameter("xT", [E, S], F16, isOutput=False)
    wq = nc.declare_dram_parameter("wq", [128, HPC * ET * D], F16, isOutput=False)
    wk = nc.declare_dram_parameter("wk", [128, ET * D], F16, isOutput=False)
    wv = nc.declare_dram_parameter("wv", [128, ET * D], F16, isOutput=False)
    wo = nc.declare_dram_parameter("wo", [128, HPC * E], F16, isOutput=False)
    cs = nc.declare_dram_parameter("cs", [128, 4 * S], F16, isOutput=False)
    any_dmask = any(m == DMASK for row in modes for m in row)
    if any_dmask:
        maskT = nc.declare_dram_parameter("maskT", [S, S], BF16, isOutput=False)
    if use_bias:
        bq = nc.declare_dram_parameter("bq", [HPC * D], F32, isOutput=False)
        bk = nc.declare_dram_parameter("bk", [D], F32, isOutput=False)
        bv = nc.declare_dram_parameter("bv", [D], F32, isOutput=False)
    out = nc.declare_dram_parameter("out", [S, E], F16, isOutput=True)

    xT_r = xT.rearrange("(t p) s -> p t s", p=128)
    cs_r = cs.rearrange("p (f s) -> p f s", s=S)

    with tile.TileContext(nc) as tc:
        with tc.tile_pool(name="const", bufs=1) as cpool:
            ones = cpool.tile([128, 128], BF16)
            nc.vector.memset(ones, 1.0)
            # warm the Exp activation table during phase-1's DMA head so the
            # first attention exp doesn't pay the 1.3us table load
            warm = cpool.tile([128, 1], F32, name="warm", tag="warm")
            nc.vector.memset(warm, 0.0)
            nc.scalar.activation(out=warm, in_=warm, func=EXP)
            # static 0/1 lower-triangle tile: dm[p, j] = 1 iff j >= p
            dmask = cpool.tile([128, 128], BF16, name="dmask", tag="dmask")
            nc.gpsimd.memset(dmask, 1.0)
            nc.gpsimd.affine_select(
                out=dmask, in_=dmask,
                pattern=[[1, 128]], base=0,
                channel_multiplier=-1,
                compare_op=IS_GE, fill=0.0)
            if use_bias:
                bq_sb = cpool.tile([128, HPC], F32)
                nc.sync.dma_start(out=bq_sb, in_=bq.rearrange("(h d) -> d h", d=128))
                bk_sb = cpool.tile([128, 1], F32)
                nc.sync.dma_start(out=bk_sb, in_=bk.rearrange("d -> d 1"))
                bv_sb = cpool.tile([128, 1], F32)
                nc.sync.dma_start(out=bv_sb, in_=bv.rearrange("d -> d 1"))

            with tc.tile_pool(name="qkv", bufs=1) as qkvpool:
                # persistent activations for the attention phase
                qr = [qkvpool.tile([128, S], F16, name=f"qr{h}", tag=f"qr{h}")
                      for h in range(HPC)]
                kr = qkvpool.tile([128, S], F16, name="kr", tag="kr")
                vT = qkvpool.tile([128, KT, 128], BF16, tag="vT")  # [k%128, kt, D]

                with tc.tile_pool(name="wpool", bufs=1) as wpool:
                    wq_sb = wpool.tile([128, HPC, ET * D], F16)
                    wk_sb = wpool.tile([128, ET * D], F16)
                    wv_sb = wpool.tile([128, ET * D], F16)
                    wo_sb = wpool.tile([128, HPC, E], F16)

                    with (
                        tc.tile_pool(name="xs", bufs=1) as xpool,
                        tc.tile_pool(name="cs", bufs=2) as cspool,
                        tc.tile_pool(name="tp", bufs=3) as tpool,
                        tc.tile_pool(name="vt", bufs=2) as vtpool,
                        # bank order matters: phase-2 pools reuse these banks
                        # in declaration order (pst<-q01, av<-q23, lp<-kv;
                        # banks 6-7 stay virgin for the o-proj pool).  sc3
                        # runs its passes q01, q23, kv so the banks drain in
                        # the order phase 2 first needs them.
                        tc.tile_pool(name="pQ01", bufs=1, space="PSUM") as ppA,
                        tc.tile_pool(name="pQ23", bufs=1, space="PSUM") as ppB,
                        tc.tile_pool(name="pKV", bufs=1, space="PSUM") as ppKV,
                    ):
                        # x tiles: 4 groups of 8 e-tiles per sc; g0/g1 double-
                        # buffered for next-sc prefetch, g2/g3 single.
                        def xtile(g):
                            return xpool.tile(
                                [128, 8, 512], F16, name=f"x{g}", tag=f"x{g}",
                                bufs=(2 if g < 2 else 1))

                        # ---- startup loads: interleave x and weight chunks
                        # per e-group so the fused sc0 pass streams (per-group
                        # DMA ~7.3us < PE ~10.2us)
                        first_x = [xtile(g) for g in range(4)]
                        for g in range(4):
                            gsl = slice(g * 1024, (g + 1) * 1024)
                            if g == 0:
                                nc.sync.dma_start(
                                    out=first_x[0][:, 0:2, :],
                                    in_=xT_r[:, 0:2, 0:512])
                                nc.sync.dma_start(out=wk_sb[:, gsl], in_=wk[:, gsl])
                                nc.sync.dma_start(out=wv_sb[:, gsl], in_=wv[:, gsl])
                                nc.sync.dma_start(
                                    out=first_x[0][:, 2:8, :],
                                    in_=xT_r[:, 2:8, 0:512])
                            else:
                                nc.sync.dma_start(
                                    out=first_x[g],
                                    in_=xT_r[:, g * 8:(g + 1) * 8, 0:512])
                                nc.sync.dma_start(out=wk_sb[:, gsl], in_=wk[:, gsl])
                                nc.sync.dma_start(out=wv_sb[:, gsl], in_=wv[:, gsl])
                            for h in range(HPC):
                                nc.sync.dma_start(
                                    out=wq_sb[:, h, gsl],
                                    in_=wq[:, h * 4096 + g * 1024:
                                           h * 4096 + (g + 1) * 1024])
                        first_cs = cspool.tile([128, 4, 512], F16, name="cs", tag="cs")
                        nc.sync.dma_start(out=first_cs, in_=cs_r[:, :, 0:512])

                        def rope(dst, acc, ct, st_, bias):
                            src = acc
                            if use_bias:
                                bsrc = tpool.tile([128, 512], F32, name="bsrc", tag="bsrc")
                                nc.vector.tensor_scalar_add(bsrc, acc, bias)
                                src = bsrc
                            tmp = tpool.tile([128, 512], F32, name="tmp", tag="tmp")
                            nc.vector.tensor_tensor(
                                out=tmp[0:64, :], in0=src[64:128, :],
                                in1=st_[0:64, :], op=MULT)
                            nc.vector.tensor_tensor(
                                out=tmp[64:128, :], in0=src[0:64, :],
                                in1=st_[64:128, :], op=MULT)
                            nc.vector.tensor_tensor(
                                out=dst, in0=src, in1=ct, op=MULT)
                            nc.vector.tensor_tensor(
                                out=dst, in0=dst, in1=tmp, op=ADD)

                        for sc in range(SC):
                            ssl = slice(sc * 512, sc * 512 + 512)
                            if sc == 0:
                                xt = first_x
                                cst = first_cs
                            else:
                                xt = [xtile(g) for g in range(4)]
                                for g in range(4):
                                    nc.sync.dma_start(
                                        out=xt[g],
                                        in_=xT_r[:, g * 8:(g + 1) * 8, ssl])
                                cst = cspool.tile([128, 4, 512], F16, name="cs", tag="cs")
                                nc.sync.dma_start(out=cst, in_=cs_r[:, :, ssl])
                            if sc == 1:
                                # prefetch wo during phase 1 (used in phase 3)
                                for h in range(HPC):
                                    nc.sync.dma_start(
                                        out=wo_sb[:, h, :],
                                        in_=wo[:, h * E:(h + 1) * E])

                            cq, sq = cst[:, 0, :], cst[:, 1, :]
                            ck, sk = cst[:, 2, :], cst[:, 3, :]

                            acc_qA = [ppA.tile([128, 512], F32, name=f"aq{h}", tag=f"aq{h}")
                                      for h in range(2)]
                            acc_qB = [ppB.tile([128, 512], F32, name=f"aq{h}", tag=f"aq{h}")
                                      for h in range(2, HPC)]
                            acc_k = ppKV.tile([128, 512], F32, name="ak", tag="ak")
                            acc_v = ppKV.tile([128, 512], F32, name="av", tag="av")
                            acc = {"k": acc_k, "v": acc_v, 0: acc_qA[0],
                                   1: acc_qA[1], 2: acc_qB[0], 3: acc_qB[1]}
                            w_of = {"k": wk_sb, "v": wv_sb}

                            def mm(key, e):
                                xr = xt[e // 8][:, e % 8, :]
                                st, sp = (e == 0), (e == ET - 1)
                                esl = slice(e * 128, e * 128 + 128)
                                wsl = (w_of[key][:, esl] if key in w_of
                                       else wq_sb[:, key, esl])
                                nc.tensor.matmul(acc[key], wsl, xr,
                                                 start=st, stop=sp)

                            def drain_kv():
                                rope(kr[:, ssl], acc_k, ck, sk,
                                     bk_sb[:, 0:1] if use_bias else None)
                                vtmp = vtpool.tile([128, 512], BF16,
                                                   name="vtmp", tag="vtmp")
                                if use_bias:
                                    nc.scalar.add(vtmp, acc_v, bv_sb[:, 0:1])
                                else:
                                    nc.scalar.copy(out=vtmp, in_=acc_v)
                                return vtmp

                            def drain_q(hs):
                                for h in hs:
                                    rope(qr[h][:, ssl], acc[h], cq, sq,
                                         bq_sb[:, h:h + 1] if use_bias else None)

                            def transposes(vtmp):
                                # HWDGE xbar transpose from SP (its seq is
                                # idle; ACT must stay free for the exps that
                                # start right at the phase-1/2 boundary)
                                for j in range(4):
                                    nc.sync.dma_start_transpose(
                                        out=vT[:, sc * 4 + j, :],
                                        in_=vtmp[:, j * 128:(j + 1) * 128])

                            if sc == 0:
                                # fused single pass: matches the interleaved
                                # per-group arrival order of x and weights
                                for e in range(ET):
                                    mm("k", e)
                                    mm("v", e)
                                    for h in range(HPC):
                                        mm(h, e)
                                vtmp = drain_kv()
                                transposes(vtmp)
                                drain_q((0, 1, 2, 3))
                            elif sc < SC - 1:
                                # 3 passes so PSUM drains overlap matmuls
                                for e in range(ET):
                                    mm("k", e)
                                    mm("v", e)
                                vtmp = drain_kv()
                                for e in range(ET):
                                    mm(0, e)
                                    mm(1, e)
                                transposes(vtmp)
                                drain_q((0, 1))
                                for e in range(ET):
                                    mm(2, e)
                                    mm(3, e)
                                drain_q((2, 3))
                            else:
                                # last chunk: drain in the order phase 2
                                # first reuses the PSUM banks (q01 -> scores,
                                # q23 -> av, kv -> lp; the kv drain is fast:
                                # rope-k on DVE + vtmp copy on ACT in parallel)
                                for e in range(ET):
                                    mm(0, e)
                                    mm(1, e)
                                drain_q((0, 1))
                                for e in range(ET):
                                    mm(2, e)
                                    mm(3, e)
                                drain_q((2, 3))
                                for e in range(ET):
                                    mm("k", e)
                                    mm("v", e)
                                vtmp = drain_kv()
                                transposes(vtmp)

                # ---------- phases 2+3: attention + output projection ----------
                with (
                    tc.tile_pool(name="mt", bufs=2) as mpool,
                    tc.tile_pool(name="ps", bufs=6) as spool,
                    tc.tile_pool(name="rl", bufs=2) as rlpool,
                    tc.tile_pool(name="ob", bufs=2) as obpool,
                    tc.tile_pool(name="osb", bufs=2) as opool,
                    tc.tile_pool(name="p2s", bufs=2, space="PSUM") as pst,
                    tc.tile_pool(name="p2a", bufs=1, space="PSUM") as pav,
                    tc.tile_pool(name="p3", bufs=2, space="PSUM") as pop,
                ):
                    oproj_pend = []
                    for qc in range(SC):
                        qsl = slice(qc * 512, qc * 512 + 512)
                        row = modes[qc]
                        mts = {}
                        for kt in range(KT):
                            if row[kt] == DMASK:
                                mi = len(mts)
                                mt = mpool.tile([128, 512], BF16,
                                                name=f"m{mi}", tag=f"m{mi}")
                                nc.sync.dma_start(
                                    out=mt,
                                    in_=maskT[kt * 128:(kt + 1) * 128, qsl])
                                mts[kt] = mt
                        # work items: (kt, q-offset, width, is_diag).  Diagonal
                        # tiles are computed only on their visible wedge
                        # [off, 512); ordered first so item 0 spans all columns
                        # (kt = 4*qc has off 0) for the PSUM start flag.
                        diag_items = []
                        full_items = []
                        for kt in range(KT):
                            if row[kt] == SKIP:
                                continue
                            if row[kt] == DIAG:
                                off = kt * 128 - qc * 512
                                diag_items.append((kt, off, 512 - off, True))
                            else:
                                full_items.append((kt, 0, 512, False))
                        diag_items.sort(key=lambda it: it[1])
                        nfull = len(full_items)
                        if diag_items and nfull >= len(diag_items):
                            # interleave so small wedge items sit between full
                            # tiles (more PE work between dependent ops); the
                            # last item is a full tile covering all columns.
                            items = []
                            fi = list(full_items)
                            for d in diag_items:
                                items.append(d)
                                items.append(fi.pop(0))
                            items.extend(fi)
                            lls = [i == len(items) - 1 for i in range(len(items))]
                        elif diag_items and nfull:
                            items = full_items + diag_items
                            lls = [it[3] for it in items]
                        elif diag_items:
                            items = diag_items
                            lls = [True] * len(items)
                        else:
                            items = full_items
                            lls = [i == len(items) - 1 for i in range(len(items))]
                        # o-proj groups from the PREVIOUS qc interleave into
                        # this qc's attention: o-proj is PE-heavy / ACT-light
                        # while attention is paced by ACT's exp, so they fill
                        # each other's engine gaps.
                        n_slots = 2 * len(items)
                        n_groups = len(oproj_pend)
                        slot_i = [0]

                        def fill(last=False):
                            if last:
                                while oproj_pend:
                                    oproj_pend.pop(0)()
                                return
                            # Bresenham spread: exactly n_groups over n_slots,
                            # evenly (front-loading leaves the attention tail
                            # unfilled and ACT-paced)
                            i = slot_i[0]
                            take = ((i + 1) * n_groups) // n_slots \
                                - (i * n_groups) // n_slots
                            slot_i[0] = i + 1
                            for _ in range(min(take, len(oproj_pend))):
                                oproj_pend.pop(0)()

                        o_sb = [None] * HPC
                        for pair in ((0, 1), (2, 3)):
                            av = {h: pav.tile([128, 512], F32,
                                              name=f"avp{h % 2}", tag=f"avp{h % 2}")
                                  for h in pair}
                            lp = {h: pav.tile([128, 512], F32,
                                              name=f"lp{h % 2}", tag=f"lp{h % 2}")
                                  for h in pair}

                            def mk_p(h, item):
                                kt, off, width, diag = item
                                stp = pst.tile([128, 512], F32, name="st", tag="st")
                                nc.tensor.matmul(
                                    stp[:, 0:width],
                                    kr[:, kt * 128:(kt + 1) * 128],
                                    qr[h][:, qc * 512 + off:qc * 512 + 512],
                                    start=True, stop=True,
                                    skip_group_check=True)
                                if row[kt] == DMASK:
                                    nc.vector.tensor_tensor(
                                        out=stp, in0=stp, in1=mts[kt], op=ADD)
                                p = spool.tile([128, 512], BF16, name="p", tag="p")
                                nc.scalar.activation(
                                    out=p[:, 0:width], in_=stp[:, 0:width],
                                    func=EXP)
                                if diag:
                                    # zero p[k_part, j] where k > q (the wedge's
                                    # leading 128 columns are the diagonal block)
                                    nc.vector.tensor_tensor(
                                        out=p[:, 0:128], in0=p[:, 0:128],
                                        in1=dmask, op=MULT)
                                return p

                            def consume(h, item, p, fl, ll):
                                kt, off, width, _ = item
                                nc.tensor.matmul(
                                    av[h][:, off:512], vT[:, kt, :],
                                    p[:, 0:width],
                                    start=fl, stop=ll, skip_group_check=True)
                                nc.tensor.matmul(
                                    lp[h][:, off:512], ones,
                                    p[:, 0:width],
                                    start=fl, stop=ll, skip_group_check=True)

                            # software-pipelined with lag 2: scores/exp for
                            # item i while AV/row-sum consume item i-2's p
                            LAG = 3 if len(items) <= 4 else 2
                            pend = []
                            for i, item in enumerate(items):
                                pend.append((i, item, [mk_p(h, item) for h in pair]))
                                fill()
                                if len(pend) > LAG:
                                    pi, pitem, ps = pend.pop(0)
                                    for h, p in zip(pair, ps):
                                        consume(h, pitem, p, pi == 0, lls[pi])
                            for pi, pitem, ps in pend:
                                for h, p in zip(pair, ps):
                                    consume(h, pitem, p, pi == 0, lls[pi])

                            for h in pair:
                                rl = rlpool.tile([128, 512], F32, name="rl", tag="rl")
                                nc.vector.reciprocal(rl, lp[h])
                                ot = opool.tile([128, 512], F16,
                                                name=f"o{h}", tag=f"o{h}")
                                nc.vector.tensor_tensor(
                                    out=ot, in0=av[h], in1=rl, op=MULT)
                                o_sb[h] = ot

                        obigs = {}

                        def oproj_group(qs4, ec,
                                        o_sb=o_sb, qc=qc, obigs=obigs):
                            q0 = qc * 512 + qs4 * 128
                            if ec == 0:
                                obigs[qs4] = obpool.tile(
                                    [128, E], F16, name="ob", tag="ob")
                            obig = obigs[qs4]
                            op = pop.tile([128, 512], F32, name="op", tag="op")
                            for h in range(HPC):
                                nc.tensor.matmul(
                                    op,
                                    o_sb[h][:, qs4 * 128:(qs4 + 1) * 128],
                                    wo_sb[:, h, ec * 512:(ec + 1) * 512],
                                    start=(h == 0), stop=(h == HPC - 1),
                                    skip_group_check=True)
                            # drain the PSUM bank on DVE (ACT is busy with
                            # exps in the interleaved attention stream)
                            dst = obig[:, ec * 512:(ec + 1) * 512]
                            last = qc == SC - 1 and qs4 == 3
                            if last and ec == ECH - 1:
                                # final tile: parallel half-copies + its own
                                # store minimize the end-of-kernel drain tail
                                nc.vector.tensor_copy(
                                    out=dst[:, 0:256], in_=op[:, 0:256])
                                nc.scalar.copy(
                                    out=dst[:, 256:512], in_=op[:, 256:512])
                            else:
                                nc.vector.tensor_copy(out=dst, in_=op)
                            if last:
                                # final row chunk: store each 512-wide tile as
                                # soon as it lands
                                nc.sync.dma_start(
                                    out=out[q0:q0 + 128,
                                            ec * 512:(ec + 1) * 512],
                                    in_=dst)
                            elif ec == 3:
                                nc.sync.dma_start(
                                    out=out[q0:q0 + 128, 0:2048],
                                    in_=obig[:, 0:2048])
                            elif ec == ECH - 1:
                                nc.sync.dma_start(
                                    out=out[q0:q0 + 128, 2048:4096],
                                    in_=obig[:, 2048:4096])

                        oproj_pend = [
                            (lambda qs4=qs4, ec=ec: oproj_group(qs4, ec))
                            for qs4 in range(4) for ec in range(ECH)]
                    # last qc's o-projection has no attention to interleave
                    fill(last=True)

    nc.finalize()
    return nc


def _host_prep(x, mask, position_ids, wq, bq, wk, bk, wv, bv, wo, bo):
    xT = np.ascontiguousarray(x.reshape(S, E).T.astype(NPF16))
    pos = position_ids.reshape(S).astype(np.float32)
    inv_freq = 1.0 / (10000.0 ** (np.arange(0, D, 2, dtype=np.float32) / D))
    freqs = np.outer(pos, inv_freq)                     # [S, D/2]
    emb = np.concatenate([freqs, freqs], axis=1)        # [S, D]
    cos = np.cos(emb).astype(np.float32)
    sin = np.sin(emb).astype(np.float32)
    sinS = sin.copy()
    sinS[:, : D // 2] *= -1.0                           # sign for partition swap
    scale = 1.0 / np.sqrt(np.float32(D))
    # cs packed [D, 4, S]: cosq*s, sinq*s, cosk, sink
    cs = np.stack([(cos * scale).T, (sinS * scale).T, cos.T, sinS.T], axis=1)
    cs = np.ascontiguousarray(cs.astype(NPF16)).reshape(128, 4 * S)

    maskF = mask.reshape(S, S)
    modes = []
    any_dmask = False
    for qc in range(SC):
        row = []
        for kt in range(KT):
            t = maskF[qc * 512:qc * 512 + 512, kt * 128:(kt + 1) * 128]  # [q, k]
            if np.all(t <= -1e8):
                row.append(SKIP)
            elif np.all(t == 0.0):
                row.append(NOMASK)
            else:
                # causal-diagonal tile? keep iff global k <= global q
                q_idx = np.arange(qc * 512, qc * 512 + 512)[:, None]
                k_idx = np.arange(kt * 128, (kt + 1) * 128)[None, :]
                causal = np.where(k_idx <= q_idx, 0.0, np.float32(-1e9))
                if np.array_equal(t, causal):
                    row.append(DIAG)
                else:
                    row.append(DMASK)
                    any_dmask = True
        if all(c == SKIP for c in row):       # fully-masked rows: keep math finite
            row = [DMASK] * KT
            any_dmask = True
        modes.append(tuple(row))
    modes = tuple(modes)

    maskT = None
    if any_dmask:
        maskT = np.ascontiguousarray(maskF.T.astype(NPBF16))

    use_bias = bool(np.any(bq) or np.any(bk) or np.any(bv))
    return xT, cs, maskT, modes, use_bias


def _pack_weights(wq, wk, wv, wo, core):
    qsl = slice(core * HPC * D, (core + 1) * HPC * D)
    ksl = slice(core * D, (core + 1) * D)
    # stationary layout [p(=e%128), h, t(=e//128), d] flattened to [128, 4*4096]
    wqp = wq[:, qsl].reshape(ET, 128, HPC, D).transpose(1, 2, 0, 3)
    wqp = np.ascontiguousarray(wqp.astype(NPF16)).reshape(128, HPC * ET * D)
    wkp = wk[:, ksl].reshape(ET, 128, D).transpose(1, 0, 2)
    wkp = np.ascontiguousarray(wkp.astype(NPF16)).reshape(128, ET * D)
    wvp = wv[:, ksl].reshape(ET, 128, D).transpose(1, 0, 2)
    wvp = np.ascontiguousarray(wvp.astype(NPF16)).reshape(128, ET * D)
    # moving layout [p(=d), h, e] flattened to [128, 4*4096]
    wop = wo[qsl, :].reshape(HPC, D, E).transpose(1, 0, 2)
    wop = np.ascontiguousarray(wop.astype(NPF16)).reshape(128, HPC * E)
    return wqp, wkp, wvp, wop


def kernel(x, mask, position_ids, wq, bq, wk, bk, wv, bv, wo, bo):
    xT, cs, maskT, modes, use_bias = _host_prep(
        x, mask, position_ids, wq, bq, wk, bk, wv, bv, wo, bo)

    key = (modes, use_bias)
    if key not in _build_cache:
        _build_cache[key] = _build(modes, use_bias)
    nc = _build_cache[key]

    in_maps = []
    for c in range(NCORES):
        wqp, wkp, wvp, wop = _pack_weights(wq, wk, wv, wo, c)
        m = {
            "xT": xT,
            "wq": wqp, "wk": wkp, "wv": wvp, "wo": wop,
            "cs": cs,
        }
        if maskT is not None:
            m["maskT"] = maskT
        if use_bias:
            qsl = slice(c * HPC * D, (c + 1) * HPC * D)
            ksl = slice(c * D, (c + 1) * D)
            m["bq"] = np.ascontiguousarray(bq[qsl]).astype(np.float32)
            m["bk"] = np.ascontiguousarray(bk[ksl]).astype(np.float32)
            m["bv"] = np.ascontiguousarray(bv[ksl]).astype(np.float32)
        in_maps.append(m)

    res = run_bass_kernel_spmd(nc, in_maps, list(range(NCORES)))
    kernel._last_results = res

    acc = res.results[0]["out"].astype(np.float32)
    for c in range(1, NCORES):
        acc = acc + res.results[c]["out"].astype(np.float32)
    acc = acc + bo[None, :]
    return acc.reshape(B, S, E).astype(np.float32)
